# revision 20
# baseline (speedup 1.0000x reference)
"""Trainium2 Bass kernel for nn_Actor_12446815223911 (dense_mlp).

Network (per row, batch B=262144):
  x [126] -> 14 groups of 9 -> shared (9->16 relu, 16->4 relu) -> concat 56
  combined = [56 | sale] (57) -> relu(57->128) -> 128->30
  mean = tanh(out[:15]), std = exp(out[15:])

Strategy: pure data parallel over 8 cores (32768 rows each), feature-major
layout (features on partitions, batch on free dim), 6 matmul passes per
512-row chunk (the M<=128 / K<=128 floor for this net). sale rides free
matmul lanes: host packs sale into spare xt rows 63/127; L1 routes it via
+-1 weights to lane 112 of each h-half (sale+ = relu(sale), sale- =
relu(-sale)); L2 forwards both to spare s-bank lanes; W3's rows for those
lanes carry +-W3[56], reconstructing sale = sale+ - sale-. This removes all
per-chunk sale DMAs. PSUM evacuations are 1024-col mega-ops spanning two
adjacent PSUM banks (h per chunk, z per chunk pair), split ACT/DVE to
balance ~72/76us under the PE's ~86us. Input DMAs are 4 big tile loads;
output DMAs are 32 strided stores. Host pre/post-transposes.
"""

import numpy as np
import ml_dtypes

B = 262144
N_CORES = 8
BPC = B // N_CORES  # 32768 rows per core
CH = 512            # rows per matmul chunk (one PSUM bank fp32)
XT = 8192           # rows per input DMA tile (16 chunks)

BF16 = ml_dtypes.bfloat16

_CACHE = {}

PROFILE = False
LAST_EXEC_NS = None
LAST_TRACE_DIR = None


def _build_nc(bpc):
    """Build the single-core Bass graph (SPMD: all 8 cores run this)."""
    import concourse.bacc as bacc
    import concourse.mybir as mybir
    import concourse.tile as tile

    f32 = mybir.dt.float32
    bf16 = mybir.dt.bfloat16
    AF = mybir.ActivationFunctionType
    ALU = mybir.AluOpType

    n_chunks = bpc // CH
    n_xt = bpc // XT

    nc = bacc.Bacc("TRN2", target_bir_lowering=False, debug=False)

    xt_d = nc.declare_dram_parameter("xt", [128, bpc], bf16, isOutput=False)
    w1_d = nc.declare_dram_parameter("w1t", [128, 113], bf16, isOutput=False)
    w2_d = nc.declare_dram_parameter("w2t", [113, 32], bf16, isOutput=False)
    w3a_d = nc.declare_dram_parameter("w3a", [128, 128], bf16, isOutput=False)
    w3b_d = nc.declare_dram_parameter("w3b", [128, 128], bf16, isOutput=False)
    w4_d = nc.declare_dram_parameter("w4t", [128, 32], bf16, isOutput=False)
    b1_d = nc.declare_dram_parameter("b1r", [113, 1], f32, isOutput=False)
    b2_d = nc.declare_dram_parameter("b2r", [125, 1], f32, isOutput=False)
    b3_d = nc.declare_dram_parameter("b3r", [128, 1], f32, isOutput=False)
    b4_d = nc.declare_dram_parameter("b4r", [128, 1], f32, isOutput=False)
    # raw staging dumps: round g2 covers chunks 8*g2..8*g2+7; partition
    # 32k+j, col g2*1024 + gg*512 + n  <->  chunk 8*g2+4*gg+k, row n,
    # out lane j (0:15 mean in o1, 15:30 std in o2). Host de-interleaves.
    o1_d = nc.declare_dram_parameter("o1", [128, bpc // 4], bf16, isOutput=True)
    o2_d = nc.declare_dram_parameter("o2", [128, bpc // 4], bf16, isOutput=True)

    with tile.TileContext(nc) as tc:
        with (
            tc.tile_pool(name="consts", bufs=1) as consts,
            tc.tile_pool(name="xt", bufs=2) as xt_pool,
            tc.tile_pool(name="work", bufs=4) as work,
            tc.tile_pool(name="outs", bufs=2) as outs,
            # PSUM budget (8 banks): h 2x2 + s 1 + z 2 + out 1
            tc.tile_pool(name="ps_h", bufs=2, space="PSUM") as ps_h,
            tc.tile_pool(name="ps_s", bufs=1, space="PSUM") as ps_s,
            tc.tile_pool(name="ps_z", bufs=1, space="PSUM") as ps_z,
            tc.tile_pool(name="ps_o", bufs=1, space="PSUM") as ps_o,
        ):
            w1 = consts.tile([128, 113], bf16)
            w2 = consts.tile([113, 32], bf16)
            w3a = consts.tile([128, 128], bf16)
            w3b = consts.tile([128, 128], bf16)
            w4 = consts.tile([128, 32], bf16)
            b1 = consts.tile([113, 1], f32)
            b2 = consts.tile([125, 1], f32)
            b3 = consts.tile([128, 1], f32)
            b4 = consts.tile([128, 1], f32)
            # spread const loads across idle engine sequencers so they fly
            # in parallel with the first xt slices (each dma_start costs
            # ~600ns of issuing-sequencer time)
            # critical path first on the sync queue: w1 -> first x slice ->
            # b1; the rest of the consts ride the gpsimd (SWDGE) queue in
            # parallel. (scalar.dma_start wedges the device - avoid.)
            nc.sync.dma_start(w1[:], w1_d[:])
            nc.gpsimd.dma_start(w2[:], w2_d[:])
            nc.gpsimd.dma_start(w3a[:], w3a_d[:])
            nc.gpsimd.dma_start(w3b[:], w3b_d[:])
            nc.gpsimd.dma_start(b3[:], b3_d[:])
            nc.gpsimd.dma_start(w4[:], w4_d[:])
            nc.gpsimd.dma_start(b2[:], b2_d[:])
            nc.gpsimd.dma_start(b4[:], b4_d[:])

            xt_t = None
            xt_next = None
            s_ps = None
            out_bank = None
            t1 = t2 = None
            for c in range(n_chunks):
                po = c % 2
                ti = c // 16
                xq = c % 16
                if c == 0:
                    # first tile in graduated slices so chunk 0 starts early;
                    # b1 (needed by the first h evac) rides between them
                    xt_t = xt_pool.tile([128, XT], bf16, tag="xt")
                    nc.sync.dma_start(xt_t[:, 0:512], xt_d[:, 0:512])
                    nc.sync.dma_start(b1[:], b1_d[:])
                    for lo, hi in ((512, 1536), (1536, 3072), (3072, 5632),
                                   (5632, 8192)):
                        nc.sync.dma_start(xt_t[:, lo:hi], xt_d[:, lo:hi])
                elif xq == 0:
                    xt_t, xt_next = xt_next, None
                if xq == 8 and ti + 1 < n_xt:
                    xt_next = xt_pool.tile([128, XT], bf16, tag="xt")
                    nc.sync.dma_start(xt_next[:],
                                      xt_d[:, (ti + 1) * XT : (ti + 2) * XT])
                xs = xt_t[:, xq * CH : (xq + 1) * CH]

                # L1: both halves into one 2-bank PSUM tile; lane 112 of
                # each half carries sale+ / sale- via +-1 weights on the
                # spare x rows 63/127.
                h_ps = ps_h.tile([113, 1024], f32, tag="h")
                nc.tensor.matmul(h_ps[:, 0:512], w1[0:64, :], xs[0:64, :],
                                 start=True, stop=True)
                nc.tensor.matmul(h_ps[:, 512:1024], w1[64:128, :],
                                 xs[64:128, :], start=True, stop=True)
                h = work.tile([113, 1024], bf16, tag="h_sb")
                if po == 0:
                    nc.vector.tensor_scalar(h[:], h_ps[:], b1[:], 0.0,
                                            ALU.add, ALU.max)
                else:
                    nc.scalar.activation(h[:], h_ps[:], AF.Relu, bias=b1[:])

                # L2: col-tiled pair into the shared 2-chunk s bank.
                # lanes: 64po+0:28 s-A | 64po+28 sale+ | 64po+32:60 s-B |
                # 64po+60 sale-
                if po == 0:
                    s_ps = ps_s.tile([128, 512], f32, tag="s")
                nc.tensor.matmul(s_ps[64 * po : 64 * po + 32, :], w2[:],
                                 h[:, 0:512], start=True, stop=True,
                                 tile_position=(0, 64 * po))
                nc.tensor.matmul(s_ps[64 * po + 32 : 64 * po + 64, :], w2[:],
                                 h[:, 512:1024], start=True, stop=True,
                                 tile_position=(0, 64 * po + 32))
                if po == 1:
                    comb = work.tile([125, 512], bf16, tag="comb")
                    nc.scalar.activation(comb[:], s_ps[0:125, :], AF.Relu,
                                         bias=b2[:])
                    # L3: chunk pair (c-1, c): K windows [0:61] / [64:125];
                    # w3a/w3b rows at the sale lanes hold +-W3[56].
                    zp = ps_z.tile([128, 1024], f32, tag="z")
                    nc.tensor.matmul(zp[:, 0:512], w3a[0:61, :],
                                     comb[0:61, :], start=True, stop=True)
                    nc.tensor.matmul(zp[:, 512:1024], w3b[64:125, :],
                                     comb[64:125, :], start=True, stop=True)
                    z = work.tile([128, 1024], bf16, tag="z_sb")
                    nc.vector.tensor_scalar(z[:], zp[:], b3[:], 0.0,
                                            ALU.add, ALU.max)
                    # L4: quad-packed out bank (4 chunks x 32 lanes)
                    qq = (c - 1) % 4
                    if qq == 0:
                        out_bank = ps_o.tile([128, 512], f32, tag="o")
                    nc.tensor.matmul(out_bank[32 * qq : 32 * qq + 32, :],
                                     w4[:], z[:, 0:512], start=True,
                                     stop=True, tile_position=(0, 32 * qq))
                    nc.tensor.matmul(out_bank[32 * qq + 32 : 32 * qq + 64, :],
                                     w4[:], z[:, 512:1024], start=True,
                                     stop=True,
                                     tile_position=(0, 32 * qq + 32))

                if c % 4 == 3:
                    g = c // 4
                    gg = g % 2
                    if gg == 0:
                        t1 = outs.tile([128, 1024], bf16, tag="t1")
                        t2 = outs.tile([128, 1024], bf16, tag="t2")
                    nc.scalar.activation(t1[:, gg * CH : (gg + 1) * CH],
                                         out_bank[:], AF.Tanh, bias=b4[:])
                    nc.scalar.activation(t2[:, gg * CH : (gg + 1) * CH],
                                         out_bank[:], AF.Exp, bias=b4[:])
                    if gg == 1:
                        g2 = g // 2
                        nc.sync.dma_start(
                            o1_d[:, g2 * 1024 : (g2 + 1) * 1024], t1[:])
                        nc.gpsimd.dma_start(
                            o2_d[:, g2 * 1024 : (g2 + 1) * 1024], t2[:])

    nc.finalize()
    return nc


def _pack_consts(W1, b1, W2, b2, W3, b3, W4, b4):
    """Host-side weight packing into the on-chip layouts."""
    f32 = np.float32
    W1 = np.asarray(W1, f32); W2 = np.asarray(W2, f32)
    W3 = np.asarray(W3, f32); W4 = np.asarray(W4, f32)
    b1 = np.asarray(b1, f32); b2 = np.asarray(b2, f32)
    b3 = np.asarray(b3, f32); b4 = np.asarray(b4, f32)

    # w1t [128, 113]: rows 0:63 A-half block-diag (+sale passthrough at
    # row 63 -> lane 112), rows 64:127 B-half (-sale at row 127)
    w1t = np.zeros((128, 113), f32)
    for half, base in ((0, 0), (1, 64)):
        for i in range(7):
            w1t[base + 9 * i: base + 9 * i + 9, 16 * i: 16 * i + 16] = W1
    w1t[63, 112] = 1.0
    w1t[127, 112] = -1.0
    # w2t [113, 32]: block-diag 7 groups + sale forward lane 112 -> col 28;
    # cols 29:32 zero so every s-bank lane gets written (no junk PSUM reads)
    w2t = np.zeros((113, 32), f32)
    for i in range(7):
        w2t[16 * i: 16 * i + 16, 4 * i: 4 * i + 4] = W2
    w2t[112, 28] = 1.0
    # s-bank lanes: 64po+0:28 sA, +28 sale+, +32:60 sB, +60 sale-
    w3a = np.zeros((128, 128), f32)   # chunk c-1 (even): K window [0:61]
    w3a[0:28] = W3[0:28]
    w3a[28] = W3[56]
    w3a[32:60] = W3[28:56]
    w3a[60] = -W3[56]
    w3b = np.zeros((128, 128), f32)   # chunk c (odd): K window [64:125]
    w3b[64:92] = W3[0:28]
    w3b[92] = W3[56]
    w3b[96:124] = W3[28:56]
    w3b[124] = -W3[56]
    w4t = np.zeros((128, 32), f32)
    w4t[:, 0:30] = W4

    b1r = np.zeros((113, 1), f32)
    b1r[0:112, 0] = np.tile(b1, 7)
    b2r = np.zeros((125, 1), f32)
    for base in (0, 32, 64, 96):
        b2r[base: base + 28, 0] = np.tile(b2, 7)
    b3r = b3[:, None]
    b4r = np.zeros((128, 1), f32)
    for k in range(4):
        b4r[32 * k: 32 * k + 30, 0] = b4
    return {
        "w1t": w1t.astype(BF16), "w2t": w2t.astype(BF16),
        "w3a": w3a.astype(BF16), "w3b": w3b.astype(BF16),
        "w4t": w4t.astype(BF16),
        "b1r": b1r, "b2r": b2r, "b3r": b3r, "b4r": b4r,
    }


def _pack_x(features_2, sale_predictions):
    """[B, 126] f32 + [B, 1] -> padded transposed [128, B] bf16 with sale
    on the spare rows 63 and 127."""
    Bn = features_2.shape[0]
    xt = np.zeros((128, Bn), dtype=BF16)
    xf = np.asarray(features_2, np.float32)
    sale = np.asarray(sale_predictions, np.float32)[:, 0]
    xt[0:63] = xf[:, 0:63].T.astype(BF16)
    xt[63] = sale.astype(BF16)
    xt[64:127] = xf[:, 63:126].T.astype(BF16)
    xt[127] = sale.astype(BF16)
    return xt


def kernel(features_2, sale_predictions, W1, b1, W2, b2, W3, b3, W4, b4):
    global LAST_EXEC_NS, LAST_TRACE_DIR
    from concourse.bass_utils import run_bass_kernel_spmd

    Bn = features_2.shape[0]
    assert Bn == B and Bn % N_CORES == 0
    bpc = Bn // N_CORES

    if bpc not in _CACHE:
        _CACHE[bpc] = _build_nc(bpc)
    nc = _CACHE[bpc]

    consts = _pack_consts(W1, b1, W2, b2, W3, b3, W4, b4)
    xt = _pack_x(features_2, sale_predictions)

    in_maps = []
    for i in range(N_CORES):
        m = dict(consts)
        m["xt"] = np.ascontiguousarray(xt[:, i * bpc: (i + 1) * bpc])
        in_maps.append(m)

    res = run_bass_kernel_spmd(
        nc, in_maps, core_ids=list(range(N_CORES)), trace=PROFILE
    )
    LAST_EXEC_NS = res.exec_time_ns
    LAST_TRACE_DIR = getattr(res, "trace_dir", None)

    action_mean = np.empty((Bn, 15), np.float32)
    action_std = np.empty((Bn, 15), np.float32)
    for i in range(N_CORES):
        # [128, bpc/4] -> [k, j32, g2, gg, n]; chunk = 8*g2 + 4*gg + k
        o1 = res.results[i]["o1"].astype(np.float32).reshape(
            4, 32, bpc // 4096, 2, 512)
        o2 = res.results[i]["o2"].astype(np.float32).reshape(
            4, 32, bpc // 4096, 2, 512)
        rows = slice(i * bpc, (i + 1) * bpc)
        action_mean[rows] = np.transpose(
            o1[:, 0:15], (2, 3, 0, 4, 1)).reshape(bpc, 15)
        action_std[rows] = np.transpose(
            o2[:, 15:30], (2, 3, 0, 4, 1)).reshape(bpc, 15)
    return (action_mean, action_std)
